# revision 1
# baseline (speedup 1.0000x reference)
"""EMA head kernel for Trainium2 (Bass/Tile), 8 NeuronCores.

Problem: alpha = clip(sigmoid(MLP(feat)), 0.01, 0.99) per (t, b);
         y[0] = r[0]; y[t] = (1-alpha[t])*y[t-1] + alpha[t]*r[t].

Sharding: time dim T=4096 split into 8 slabs of 512 (all B=256 per core).
Each core computes, for its slab, the local affine-scan pieces
    z[t] = A[t]*z[t-1] + Bv[t]   (z[-1] = 0),   A = 1-alpha, Bv = alpha*r
    P[t] = A[t]*P[t-1]           (P[-1] = 1)
and the host stitches slabs with   y = z + P * carry,  carry' = y[-1].
carry_0 = r[0] reproduces y[0] = r[0] exactly: a*r + (1-a)*r = r.

v11: feat is quantized to fp8(e4m3) and pre-transposed to [f, t*b] on
the host; the device streams it with 4MB contiguous HWDGE DMAs
(~410 GB/s).  One 128x128 matmul per (t, b-half) column block against
W1 (fp8, FWL), with rank-1 PE matmuls folding b1 into each PSUM bank.
h is collected 64 t-steps per 2-bank PSUM tile; epilogue per block is
1 ACT relu + 1 mul (alternating DVE/GpSimd) + 1 DVE reduce.  The
alpha->A/Bv->scan tail runs per 128-t segment: sigmoid on ACT,
clip/A/Bv on GpSimd (own queue), chained scans on DVE, z/P DMA out per
segment.  First/last feat DMAs are split in half to shorten the
pipeline lead-in/drain.
"""

import numpy as np

T, B, FEAT, HID = 4096, 256, 128, 16
NCORES = 8
TLOC = T // NCORES  # 512
NH = 2              # batch halves of 128
CH = 32768          # (t,b) columns per feat chunk (128 t-steps, 4 MB fp8)
NCHUNK = TLOC * B // CH  # 4
TCH = CH // B       # 128 t-steps per chunk
TBLK = 64           # t-steps per PSUM block (2 banks)

_CACHE = {}


def _build_program():
    import concourse.bacc as bacc
    import concourse.bass as bass
    import concourse.tile as tile
    from concourse import mybir

    fp32 = mybir.dt.float32
    fp16 = mybir.dt.float16
    fp8 = mybir.dt.float8e4
    AF = mybir.ActivationFunctionType
    OP = mybir.AluOpType

    nc = bacc.Bacc("TRN2", target_bir_lowering=False, debug=False,
                   num_devices=NCORES)

    feat_d = nc.dram_tensor("feat", [FEAT, TLOC * B], fp8, kind="ExternalInput")
    rt_d = nc.dram_tensor("rt", [NH, 128, TLOC], fp32, kind="ExternalInput")
    w1_d = nc.dram_tensor("w1", [FEAT, HID], fp8, kind="ExternalInput")
    b1r_d = nc.dram_tensor("b1r", [1, 32 * HID], fp16, kind="ExternalInput")
    w2rep_d = nc.dram_tensor("w2rep", [128, TBLK * HID], fp16,
                             kind="ExternalInput")
    b2col_d = nc.dram_tensor("b2col", [128, 1], fp32, kind="ExternalInput")
    z_d = nc.dram_tensor("z", [NH, 128, TLOC], fp32, kind="ExternalOutput")
    p_d = nc.dram_tensor("p", [NH, 128, TLOC], fp32, kind="ExternalOutput")

    with tile.TileContext(nc) as tc:
        with (
            tc.tile_pool(name="singles", bufs=1) as singles,
            tc.tile_pool(name="featin", bufs=3) as featin,
            tc.tile_pool(name="hps", bufs=2, space="PSUM") as hps,
            tc.tile_pool(name="hwork", bufs=2) as hwork,
        ):
            # ------------- constants / small inputs (all HWDGE) -------------
            w1_sb = singles.tile([128, HID], fp8)
            nc.scalar.dma_start(w1_sb, w1_d[:, :])
            b1row = singles.tile([1, 32 * HID], fp16)
            nc.scalar.dma_start(b1row, b1r_d[:, :])
            ones1 = singles.tile([1, 128], fp16)
            nc.vector.memset(ones1, 1.0)
            w2rep = singles.tile([128, TBLK, HID], fp16)
            nc.scalar.dma_start(
                w2rep, w2rep_d[:, :].rearrange("p (t h) -> p t h", h=HID))
            b2col = singles.tile([128, 1], fp32)
            nc.scalar.dma_start(b2col, b2col_d[:, :])
            ones_sb = singles.tile([128, TLOC], fp32)
            nc.vector.memset(ones_sb, 1.0)

            rT = [singles.tile([128, TLOC], fp32, tag=f"rT{h}", name=f"rT{h}")
                  for h in range(NH)]
            for h in range(NH):
                nc.scalar.dma_start(rT[h], rt_d[h])

            # per-half alpha_pre accumulators [128 b, t] + tail tiles
            apre = [singles.tile([128, TLOC], fp32, tag=f"apre{h}",
                                 name=f"apre{h}") for h in range(NH)]
            alpha = [singles.tile([128, TLOC], fp32, tag=f"alpha{h}",
                                  name=f"alpha{h}") for h in range(NH)]
            A_sb = [singles.tile([128, TLOC], fp32, tag=f"A{h}",
                                 name=f"A{h}") for h in range(NH)]
            Bv = [singles.tile([128, TLOC], fp32, tag=f"Bv{h}",
                               name=f"Bv{h}") for h in range(NH)]
            z_sb = [singles.tile([128, TLOC], fp32, tag=f"z{h}",
                                 name=f"z{h}") for h in range(NH)]
            p_sb = [singles.tile([128, TLOC], fp32, tag=f"p{h}",
                                 name=f"p{h}") for h in range(NH)]

            def tail_segment(lo, hi):
                """alpha -> clip -> A, Bv (GpSimd) -> chained scans (DVE)."""
                sl = slice(lo, hi)
                for h in range(NH):
                    nc.scalar.activation(alpha[h][:, sl], apre[h][:, sl],
                                         AF.Sigmoid, bias=b2col)
                    nc.vector.tensor_scalar(alpha[h][:, sl], alpha[h][:, sl],
                                            0.01, 0.99, op0=OP.max, op1=OP.min)
                    nc.vector.tensor_scalar(A_sb[h][:, sl], alpha[h][:, sl],
                                            -1.0, 1.0, op0=OP.mult, op1=OP.add)
                    nc.vector.tensor_mul(Bv[h][:, sl], alpha[h][:, sl],
                                         rT[h][:, sl])
                    nc.vector.tensor_tensor_scan(
                        z_sb[h][:, sl], A_sb[h][:, sl], Bv[h][:, sl],
                        0.0 if lo == 0 else z_sb[h][:, lo - 1:lo],
                        op0=OP.mult, op1=OP.add)
                    nc.vector.tensor_tensor_scan(
                        p_sb[h][:, sl], A_sb[h][:, sl], ones_sb[:, sl],
                        1.0 if lo == 0 else p_sb[h][:, lo - 1:lo],
                        op0=OP.mult, op1=OP.mult)
                    nc.sync.dma_start(z_d[h, :, sl], z_sb[h][:, sl])
                    nc.sync.dma_start(p_d[h, :, sl], p_sb[h][:, sl])

            # ---------------- main feat pipeline ----------------
            mul_parity = 0
            for k in range(NCHUNK):
                ft = featin.tile([128, CH], fp8, tag="ft")
                if k == 0 or k == NCHUNK - 1:
                    nsub = 4 if k == NCHUNK - 1 else 2
                    for q in range(nsub):
                        nc.sync.dma_start(
                            ft[:, q * (CH // nsub):(q + 1) * (CH // nsub)],
                            feat_d[:, k * CH + q * (CH // nsub):
                                   k * CH + (q + 1) * (CH // nsub)])
                else:
                    nc.sync.dma_start(ft, feat_d[:, k * CH:(k + 1) * CH])
                for blk in range(TCH // TBLK):
                    hbank = [hps.tile([128, TBLK, HID], fp32, tag=f"h{h}",
                                      name=f"hbank{h}") for h in range(NH)]
                    for h in range(NH):
                        # rank-1 bias: psum[:, t, hid] = b1[hid] (per bank)
                        for q in range(2):
                            nc.tensor.matmul(
                                hbank[h][:, q * 32:(q + 1) * 32, :],
                                ones1, b1row,
                                start=True, stop=False, skip_group_check=True)
                    for tt in range(TBLK):
                        col = (blk * TBLK + tt) * B
                        for h in range(NH):
                            nc.tensor.matmul(
                                hbank[h][:, tt, :],
                                ft[:, col + h * 128:col + (h + 1) * 128],
                                w1_sb, start=False, stop=True,
                                skip_group_check=True)
                    t0 = k * TCH + blk * TBLK
                    for h in range(NH):
                        hrelu = hwork.tile([128, TBLK, HID], fp16, tag="hrelu")
                        nc.scalar.activation(hrelu, hbank[h], AF.Relu)
                        hw = hwork.tile([128, TBLK, HID], fp16, tag="hw")
                        if mul_parity == 0:
                            nc.vector.tensor_mul(hw, hrelu, w2rep)
                        else:
                            nc.gpsimd.tensor_mul(hw, hrelu, w2rep)
                        mul_parity ^= 1
                        nc.vector.tensor_reduce(
                            apre[h][:, t0:t0 + TBLK],
                            hw, axis=mybir.AxisListType.X, op=OP.add)
                if k < NCHUNK - 1:
                    tail_segment(k * TCH, (k + 1) * TCH)
            tail_segment((NCHUNK - 1) * TCH, NCHUNK * TCH)

    nc.finalize()
    return nc


def _get_program():
    if "nc" not in _CACHE:
        _CACHE["nc"] = _build_program()
    return _CACHE["nc"]


def _host_in_maps(r, feat, W1, b1, W2, b2):
    import ml_dtypes
    W1 = np.asarray(W1, dtype=np.float32).astype(ml_dtypes.float8_e4m3)
    b1 = np.asarray(b1, dtype=np.float32).reshape(HID)
    W2 = np.asarray(W2, dtype=np.float32).reshape(HID)
    b2 = np.asarray(b2, dtype=np.float32).reshape(1)
    b1r = np.ascontiguousarray(
        np.tile(b1.astype(np.float16), 32)[None, :])
    w2rep = np.ascontiguousarray(np.broadcast_to(
        np.tile(W2.astype(np.float16), TBLK)[None, :], (128, TBLK * HID)))
    b2col = np.ascontiguousarray(np.broadcast_to(b2[None, :], (128, 1)))
    feat16 = np.ascontiguousarray(
        feat.reshape(T * B, FEAT)).astype(np.float16)
    r2 = r[:, :, 0]
    in_maps = []
    BL = 4096  # transpose block: 1 MB input window, L2-resident
    for c in range(NCORES):
        base = c * TLOC * B
        featT = np.empty((FEAT, TLOC * B), np.float16)
        for j in range(0, TLOC * B, BL):
            featT[:, j:j + BL] = feat16[base + j:base + j + BL, :].T
        featT = featT.astype(ml_dtypes.float8_e4m3)
        rt = np.ascontiguousarray(
            r2[c * TLOC:(c + 1) * TLOC, :].T).reshape(NH, 128, TLOC)
        in_maps.append({
            "feat": featT,
            "rt": rt,
            "w1": W1, "b1r": b1r, "w2rep": w2rep, "b2col": b2col,
        })
    return in_maps


def kernel(r, feat, W1, b1, W2, b2, _run_kwargs=None, _return_results=False):
    from concourse.bass_utils import run_bass_kernel_spmd

    r = np.asarray(r, dtype=np.float32)
    feat = np.asarray(feat, dtype=np.float32)

    nc = _get_program()
    in_maps = _host_in_maps(r, feat, W1, b1, W2, b2)

    kw = _run_kwargs or {}
    res = run_bass_kernel_spmd(nc, in_maps, core_ids=list(range(NCORES)), **kw)

    # host stitch: y = z + P*carry per slab, carry chain across slabs
    y = np.empty((T, B), dtype=np.float32)
    carry = r[0, :, 0].astype(np.float32)
    for c in range(NCORES):
        zc = res.results[c]["z"].transpose(2, 0, 1).reshape(TLOC, B)
        pc = res.results[c]["p"].transpose(2, 0, 1).reshape(TLOC, B)
        y_slab = zc + pc * carry[None, :]
        carry = y_slab[-1]
        y[c * TLOC:(c + 1) * TLOC] = y_slab
    out = y[:, :, None]
    if _return_results:
        return out, res
    return out



# revision 6
# speedup vs baseline: 1.1928x; 1.1928x over previous
"""EMA head kernel for Trainium2 (Bass/Tile), 8 NeuronCores.

Problem: alpha = clip(sigmoid(MLP(feat)), 0.01, 0.99) per (t, b);
         y[0] = r[0]; y[t] = (1-alpha[t])*y[t-1] + alpha[t]*r[t].

Sharding v12: 2 batch-halves x 4 time-quarters.  Each core handles
128 b-lanes (full partition dim) x 1024 t.  Per-core affine-scan
pieces z/P as in v11; host stitches slabs with y = z + P*carry.

Math: W2 and b1 are folded into the fp8 layer-1 weights
    w1f[:, h'] = S * w2[h'] * W1[:, h']   (S = 16, h' sign-sorted)
so that sum_h w2*relu(x+b1) = (sum_h' maxmin(y_h', c_h'))/S + sum w2*b1
with y = feat @ w1f, c = -S*w2*b1, max for w2>=0 cols and min for
w2<0 cols.  This removes the bias matmuls, the relu pass, and the
w2 multiply: the epilogue is one max+min pair (DVE/Pool alternating)
plus one DVE reduce per 64-t PSUM block.  The constant sum w2*b1 is
folded into the sigmoid bias; S into the sigmoid input scale.

feat is fp8(e4m3), host-pretransposed to [f, t*128b]; z/p/A/Bv/alpha/
apre/rt are fp16 (scan state is fp32 internally).  feat streams on the
sync queue; z/p outputs go out on the scalar queue.
"""

import numpy as np

T, B, FEAT, HID = 4096, 256, 128, 16
NCORES = 8
TQ, BH = 4, 2        # time-quarters x batch-halves
TLOC = T // TQ       # 1024 t per core
BLOC = B // BH       # 128 b per core
S = 16.0             # w1 fold scale
CH_T = 128           # t-steps per feat chunk (2 MB fp8)
NCHUNK = TLOC // CH_T  # 8
TBLK = 64            # t-steps per PSUM block (2 banks)
SEG = 256            # t-steps per scan/tail segment

_CACHE = {}


def _build_program(npos):
    import concourse.bacc as bacc
    import concourse.tile as tile
    from concourse import mybir

    fp32 = mybir.dt.float32
    fp16 = mybir.dt.float16
    fp8 = mybir.dt.float8e4
    AF = mybir.ActivationFunctionType
    OP = mybir.AluOpType

    nc = bacc.Bacc("TRN2", target_bir_lowering=False, debug=False,
                   num_devices=NCORES)

    feat_d = nc.dram_tensor("feat", [FEAT, TLOC * BLOC], fp8,
                            kind="ExternalInput")
    rt_d = nc.dram_tensor("rt", [BLOC, TLOC], fp16, kind="ExternalInput")
    w1_d = nc.dram_tensor("w1", [FEAT, HID], fp8, kind="ExternalInput")
    crep_d = nc.dram_tensor("crep", [BLOC, TBLK * HID], fp16,
                            kind="ExternalInput")
    b2col_d = nc.dram_tensor("b2col", [BLOC, 1], fp32, kind="ExternalInput")
    z_d = nc.dram_tensor("z", [BLOC, TLOC], fp16, kind="ExternalOutput")
    p_d = nc.dram_tensor("p", [BLOC, TLOC], fp16, kind="ExternalOutput")

    with tile.TileContext(nc) as tc:
        with (
            tc.tile_pool(name="singles", bufs=1) as singles,
            tc.tile_pool(name="featin", bufs=4) as featin,
            tc.tile_pool(name="hps", bufs=3, space="PSUM") as hps,
            tc.tile_pool(name="hwork", bufs=3) as hwork,
        ):
            # first feat chunk DMA before anything else (shortest lead-in)
            ft0 = featin.tile([128, CH_T * BLOC], fp8, tag="ft")
            NSUB0 = 4
            sub0 = CH_T * BLOC // NSUB0
            for q in range(NSUB0):
                nc.sync.dma_start(ft0[:, q * sub0:(q + 1) * sub0],
                                  feat_d[:, q * sub0:(q + 1) * sub0])

            # constants / small inputs on the scalar queue
            w1_sb = singles.tile([128, HID], fp8)
            nc.scalar.dma_start(w1_sb, w1_d[:, :])
            crep_sb = singles.tile([128, TBLK, HID], fp16)
            nc.scalar.dma_start(
                crep_sb, crep_d[:, :].rearrange("p (t h) -> p t h", h=HID))
            b2col = singles.tile([128, 1], fp32)
            nc.scalar.dma_start(b2col, b2col_d[:, :])
            rt_sb = singles.tile([128, TLOC], fp16)
            nc.scalar.dma_start(rt_sb, rt_d[:, :])
            ones_sb = singles.tile([128, TLOC], fp16)
            nc.vector.memset(ones_sb, 1.0)

            apre = singles.tile([128, TLOC], fp16, name="apre")
            alpha = singles.tile([128, TLOC], fp16, name="alpha")
            A_sb = singles.tile([128, TLOC], fp16, name="A")
            Bv = singles.tile([128, TLOC], fp16, name="Bv")
            z_sb = singles.tile([128, TLOC], fp16, name="z")
            p_sb = singles.tile([128, TLOC], fp16, name="p")

            def tail_segment(lo, hi):
                sl = slice(lo, hi)
                nc.scalar.activation(alpha[:, sl], apre[:, sl], AF.Sigmoid,
                                     bias=b2col, scale=1.0 / S)
                nc.vector.tensor_scalar(alpha[:, sl], alpha[:, sl],
                                        0.01, 0.99, op0=OP.max, op1=OP.min)
                nc.gpsimd.tensor_scalar(A_sb[:, sl], alpha[:, sl],
                                        -1.0, 1.0, op0=OP.mult, op1=OP.add)
                nc.gpsimd.tensor_mul(Bv[:, sl], alpha[:, sl], rt_sb[:, sl])
                nc.vector.tensor_tensor_scan(
                    z_sb[:, sl], A_sb[:, sl], Bv[:, sl],
                    0.0 if lo == 0 else z_sb[:, lo - 1:lo],
                    op0=OP.mult, op1=OP.add)
                nc.vector.tensor_tensor_scan(
                    p_sb[:, sl], A_sb[:, sl], ones_sb[:, sl],
                    1.0 if lo == 0 else p_sb[:, lo - 1:lo],
                    op0=OP.mult, op1=OP.mult)
                nc.scalar.dma_start(z_d[:, sl], z_sb[:, sl])
                nc.scalar.dma_start(p_d[:, sl], p_sb[:, sl])

            blk_idx = 0
            for k in range(NCHUNK):
                if k == 0:
                    ft = ft0
                else:
                    ft = featin.tile([128, CH_T * BLOC], fp8, tag="ft")
                    sub = CH_T * BLOC // 2
                    for q in range(2):
                        nc.sync.dma_start(
                            ft[:, q * sub:(q + 1) * sub],
                            feat_d[:, k * CH_T * BLOC + q * sub:
                                   k * CH_T * BLOC + (q + 1) * sub])
                for blk in range(CH_T // TBLK):
                    hbank = hps.tile([128, TBLK, HID], fp32, name="hbank")
                    for tt in range(TBLK):
                        col = (blk * TBLK + tt) * BLOC
                        nc.tensor.matmul(
                            hbank[:, tt, :], ft[:, col:col + BLOC], w1_sb,
                            start=True, stop=True, skip_group_check=True)
                    # ACT converts PSUM fp32 -> SBUF fp16 (Pool cannot touch
                    # PSUM, and fp16 operands put the DVE ops in 2x mode);
                    # DVE then does the shifted-threshold max/min in place.
                    hw = hwork.tile([128, TBLK, HID], fp16, tag="hw")
                    nc.scalar.activation(hw, hbank, AF.Copy)
                    if npos > 0:
                        nc.vector.tensor_tensor(
                            hw[:, :, :npos], hw[:, :, :npos],
                            crep_sb[:, :, :npos], op=OP.max)
                    if npos < HID:
                        nc.vector.tensor_tensor(
                            hw[:, :, npos:], hw[:, :, npos:],
                            crep_sb[:, :, npos:], op=OP.min)
                    t0 = k * CH_T + blk * TBLK
                    with nc.allow_low_precision(
                            "apre fp16 validated vs numpy, 16-elem sums"):
                        nc.vector.tensor_reduce(
                            apre[:, t0:t0 + TBLK], hw,
                            axis=mybir.AxisListType.X, op=OP.add)
                    blk_idx += 1
                if k % 2 == 1 and k < NCHUNK - 1:
                    tail_segment((k - 1) * CH_T, (k + 1) * CH_T)
            tail_segment(TLOC - SEG, TLOC)

    nc.finalize()
    return nc


def _get_program(npos):
    key = ("nc", npos)
    if key not in _CACHE:
        _CACHE[key] = _build_program(npos)
    return _CACHE[key]


def _host_prep(r, feat, W1, b1, W2, b2):
    import ml_dtypes
    W1 = np.asarray(W1, dtype=np.float32)
    b1 = np.asarray(b1, dtype=np.float32).reshape(HID)
    W2 = np.asarray(W2, dtype=np.float32).reshape(HID)
    b2 = float(np.asarray(b2, dtype=np.float32).reshape(1)[0])

    perm = np.argsort(W2 < 0, kind="stable")
    w2s, b1s = W2[perm], b1[perm]
    npos = int((w2s >= 0).sum())

    w1f = (S * w2s[None, :] * W1[:, perm]).astype(ml_dtypes.float8_e4m3)
    c = (-S * w2s * b1s).astype(np.float16)
    crep = np.ascontiguousarray(np.broadcast_to(
        np.tile(c, TBLK)[None, :], (BLOC, TBLK * HID)))
    b2p = b2 + float((W2 * b1).sum())
    b2col = np.full((BLOC, 1), b2p, dtype=np.float32)

    feat16 = np.ascontiguousarray(
        feat.reshape(T * B, FEAT)).astype(np.float16)
    feat16 = feat16.reshape(T, B, FEAT)
    r2 = r[:, :, 0]

    in_maps = []
    BLT = 32  # t rows per transpose block (1 MB window)
    for c_id in range(NCORES):
        tq, hb = divmod(c_id, BH)
        tsl = slice(tq * TLOC, (tq + 1) * TLOC)
        bsl = slice(hb * BLOC, (hb + 1) * BLOC)
        fblk = feat16[tsl, bsl, :]  # [1024, 128, 128] (t, b, f)
        featT = np.empty((FEAT, TLOC * BLOC), np.float16)
        for j in range(0, TLOC, BLT):
            featT[:, j * BLOC:(j + BLT) * BLOC] = (
                fblk[j:j + BLT].reshape(BLT * BLOC, FEAT).T)
        featT = featT.astype(ml_dtypes.float8_e4m3)
        rt = np.ascontiguousarray(r2[tsl, bsl].T).astype(np.float16)
        in_maps.append({
            "feat": featT, "rt": rt,
            "w1": w1f, "crep": crep, "b2col": b2col,
        })
    return in_maps, npos


def kernel(r, feat, W1, b1, W2, b2, _run_kwargs=None, _return_results=False):
    from concourse.bass_utils import run_bass_kernel_spmd

    r = np.asarray(r, dtype=np.float32)
    feat = np.asarray(feat, dtype=np.float32)

    in_maps, npos = _host_prep(r, feat, W1, b1, W2, b2)
    nc = _get_program(npos)

    kw = _run_kwargs or {}
    res = run_bass_kernel_spmd(nc, in_maps, core_ids=list(range(NCORES)), **kw)

    # host stitch: y = z + P*carry per slab, carry chain across t-quarters
    y = np.empty((T, B), dtype=np.float32)
    for hb in range(BH):
        bsl = slice(hb * BLOC, (hb + 1) * BLOC)
        carry = r[0, bsl, 0].astype(np.float32)
        for tq in range(TQ):
            c_id = tq * BH + hb
            zc = res.results[c_id]["z"].astype(np.float32).T  # [TLOC, BLOC]
            pc = res.results[c_id]["p"].astype(np.float32).T
            y_slab = zc + pc * carry[None, :]
            carry = y_slab[-1]
            y[tq * TLOC:(tq + 1) * TLOC, bsl] = y_slab
    out = y[:, :, None]
    if _return_results:
        return out, res
    return out


# revision 8
# speedup vs baseline: 1.2310x; 1.0320x over previous
"""EMA head kernel for Trainium2 (Bass/Tile), 8 NeuronCores.

Problem: alpha = clip(sigmoid(MLP(feat)), 0.01, 0.99) per (t, b);
         y[0] = r[0]; y[t] = (1-alpha[t])*y[t-1] + alpha[t]*r[t].

Sharding v13: 2 batch-halves x 4 time-quarters.  Each core handles
128 b-lanes (full partition dim) x 1024 t.  Per-core affine-scan
pieces z/P; host stitches slabs with y = z + P*carry.

Math: W2, b1 and the layer-1 bias are all folded away on the host:
  w1f[:, h'] = S * |w2[h']| * W1[:, h']   (S = 16, h' sign-sorted)
  feat' = feat + beta  where  w1f.T beta = S*|w2|*b1  (least-norm)
so that  sum_h w2*relu(x+b1) = (sum_pos relu(y) - sum_neg relu(y))/S
with y = feat' @ w1f.  The per-block epilogue is then one ACT Relu
(PSUM fp32 -> SBUF fp16) and two DVE reduces (pos/neg column groups
into apreP/apreN); a per-segment subtract + sigmoid(bias=b2,
scale=1/S) recovers alpha.

feat is fp8(e4m3), host-pretransposed to [f, t*128b]; everything
vector-side is fp16 (scan state is fp32 internally).  feat streams on
the sync queue; z/p go out per segment, issues deferred on the sync
queue so they never block feat chunk issues.
"""

import numpy as np

T, B, FEAT, HID = 4096, 256, 128, 16
NCORES = 8
TQ, BH = 4, 2        # time-quarters x batch-halves
TLOC = T // TQ       # 1024 t per core
BLOC = B // BH       # 128 b per core
S = 16.0             # w1 fold scale
CH_T = 128           # t-steps per feat chunk (2 MB fp8)
NCHUNK = TLOC // CH_T  # 8
TBLK = 64            # t-steps per PSUM block (2 banks)
SEG = 256            # t-steps per scan/tail segment

_CACHE = {}


def _build_program(npos):
    import concourse.bacc as bacc
    import concourse.tile as tile
    from concourse import mybir

    fp32 = mybir.dt.float32
    fp16 = mybir.dt.float16
    fp8 = mybir.dt.float8e4
    AF = mybir.ActivationFunctionType
    OP = mybir.AluOpType

    nc = bacc.Bacc("TRN2", target_bir_lowering=False, debug=False,
                   num_devices=NCORES)

    feat_d = nc.dram_tensor("feat", [FEAT, TLOC * BLOC], fp8,
                            kind="ExternalInput")
    rt_d = nc.dram_tensor("rt", [BLOC, TLOC], fp16, kind="ExternalInput")
    w1_d = nc.dram_tensor("w1", [FEAT, HID], fp8, kind="ExternalInput")
    b2col_d = nc.dram_tensor("b2col", [BLOC, 1], fp32, kind="ExternalInput")
    z_d = nc.dram_tensor("z", [BLOC, TLOC], fp16, kind="ExternalOutput")
    p_d = nc.dram_tensor("p", [BLOC, TLOC], fp16, kind="ExternalOutput")

    with tile.TileContext(nc) as tc:
        with (
            tc.tile_pool(name="singles", bufs=1) as singles,
            tc.tile_pool(name="featin", bufs=5) as featin,
            tc.tile_pool(name="hps", bufs=3, space="PSUM") as hps,
            tc.tile_pool(name="hwork", bufs=3) as hwork,
        ):
            # first feat chunk DMA before anything else (shortest lead-in);
            # tapered sub-splits so the first 64-t block is ready earliest
            ft0 = featin.tile([128, CH_T * BLOC], fp8, tag="ft")
            for lo, hi in ((0, 16), (16, 32), (32, 64), (64, 128)):
                nc.sync.dma_start(ft0[:, lo * BLOC:hi * BLOC],
                                  feat_d[:, lo * BLOC:hi * BLOC])

            # constants / small inputs on the scalar queue
            w1_sb = singles.tile([128, HID], fp8)
            nc.scalar.dma_start(w1_sb, w1_d[:, :])
            b2col = singles.tile([128, 1], fp32)
            nc.scalar.dma_start(b2col, b2col_d[:, :])
            rt_sb = singles.tile([128, TLOC], fp16)
            nc.scalar.dma_start(rt_sb, rt_d[:, :])
            ones_sb = singles.tile([128, TLOC], fp16)
            nc.vector.memset(ones_sb, 1.0)

            apreP = singles.tile([128, TLOC], fp16, name="apreP")
            apreN = singles.tile([128, TLOC], fp16, name="apreN")
            if npos == HID:
                nc.vector.memset(apreN, 0.0)
            if npos == 0:
                nc.vector.memset(apreP, 0.0)
            alpha = singles.tile([128, TLOC], fp16, name="alpha")
            A_sb = singles.tile([128, TLOC], fp16, name="A")
            Bv = singles.tile([128, TLOC], fp16, name="Bv")
            z_sb = singles.tile([128, TLOC], fp16, name="z")
            p_sb = singles.tile([128, TLOC], fp16, name="p")

            zp_out = []  # deferred z/p DMAs (sync queue, after ft issues)

            def tail_segment(lo, hi):
                sl = slice(lo, hi)
                nc.vector.tensor_sub(apreP[:, sl], apreP[:, sl],
                                     apreN[:, sl])
                nc.scalar.activation(alpha[:, sl], apreP[:, sl], AF.Sigmoid,
                                     bias=b2col, scale=1.0 / S)
                nc.vector.tensor_scalar(alpha[:, sl], alpha[:, sl],
                                        0.01, 0.99, op0=OP.max, op1=OP.min)
                nc.gpsimd.tensor_scalar(A_sb[:, sl], alpha[:, sl],
                                        -1.0, 1.0, op0=OP.mult, op1=OP.add)
                nc.gpsimd.tensor_mul(Bv[:, sl], alpha[:, sl], rt_sb[:, sl])
                nc.vector.tensor_tensor_scan(
                    z_sb[:, sl], A_sb[:, sl], Bv[:, sl],
                    0.0 if lo == 0 else z_sb[:, lo - 1:lo],
                    op0=OP.mult, op1=OP.add)
                nc.vector.tensor_tensor_scan(
                    p_sb[:, sl], A_sb[:, sl], ones_sb[:, sl],
                    1.0 if lo == 0 else p_sb[:, lo - 1:lo],
                    op0=OP.mult, op1=OP.mult)
                zp_out.append(sl)

            t_idx = 0
            for k in range(NCHUNK):
                if k == 0:
                    ft = ft0
                else:
                    ft = featin.tile([128, CH_T * BLOC], fp8, tag="ft")
                    sub = CH_T * BLOC // 2
                    for q in range(2):
                        nc.sync.dma_start(
                            ft[:, q * sub:(q + 1) * sub],
                            feat_d[:, k * CH_T * BLOC + q * sub:
                                   k * CH_T * BLOC + (q + 1) * sub])
                for blk in range(CH_T // TBLK):
                    hbank = hps.tile([128, TBLK, HID], fp32, name="hbank")
                    for tt in range(TBLK):
                        col = (blk * TBLK + tt) * BLOC
                        nc.tensor.matmul(
                            hbank[:, tt, :], ft[:, col:col + BLOC], w1_sb,
                            start=True, stop=True, skip_group_check=True)
                    # ACT applies relu while converting PSUM fp32 -> fp16;
                    # DVE then sums the pos / neg column groups.
                    hw = hwork.tile([128, TBLK, HID], fp16, tag="hw")
                    nc.scalar.activation(hw, hbank, AF.Relu)
                    t0 = k * CH_T + blk * TBLK
                    with nc.allow_low_precision(
                            "fp16 apre validated vs numpy, 16-elem sums"):
                        if npos > 0:
                            nc.vector.tensor_reduce(
                                apreP[:, t0:t0 + TBLK], hw[:, :, :npos],
                                axis=mybir.AxisListType.X, op=OP.add)
                        if npos < HID:
                            nc.vector.tensor_reduce(
                                apreN[:, t0:t0 + TBLK], hw[:, :, npos:],
                                axis=mybir.AxisListType.X, op=OP.add)
                    t_idx += 1
                if k % 2 == 1 and k < NCHUNK - 1:
                    tail_segment((k - 1) * CH_T, (k + 1) * CH_T)
            # deferred z/p output DMAs: on the sync queue, after every feat
            # chunk issue so they can never stall the feat stream
            for sl in zp_out:
                nc.sync.dma_start(z_d[:, sl], z_sb[:, sl])
                nc.sync.dma_start(p_d[:, sl], p_sb[:, sl])
            tail_segment(TLOC - SEG, TLOC)
            sl = zp_out[-1]
            nc.sync.dma_start(z_d[:, sl], z_sb[:, sl])
            nc.sync.dma_start(p_d[:, sl], p_sb[:, sl])

    nc.finalize()
    return nc


def _get_program(npos):
    key = ("nc", npos)
    if key not in _CACHE:
        _CACHE[key] = _build_program(npos)
    return _CACHE[key]


def _host_prep(r, feat, W1, b1, W2, b2):
    import ml_dtypes
    W1 = np.asarray(W1, dtype=np.float32)
    b1 = np.asarray(b1, dtype=np.float32).reshape(HID)
    W2 = np.asarray(W2, dtype=np.float32).reshape(HID)
    b2 = float(np.asarray(b2, dtype=np.float32).reshape(1)[0])

    perm = np.argsort(W2 < 0, kind="stable")
    w2s, b1s = W2[perm], b1[perm]
    npos = int((w2s >= 0).sum())

    w1f8 = (S * np.abs(w2s)[None, :] * W1[:, perm]).astype(
        ml_dtypes.float8_e4m3)
    w1fq = w1f8.astype(np.float64)  # dequantized, for the bias solve
    d = (S * np.abs(w2s) * b1s).astype(np.float64)
    beta = np.linalg.lstsq(w1fq.T, d, rcond=None)[0].astype(np.float32)

    b2col = np.full((BLOC, 1), b2, dtype=np.float32)

    r2 = r[:, :, 0]
    in_maps = []
    BLT = 32  # t rows per transpose block (1 MB window)
    for c_id in range(NCORES):
        tq, hb = divmod(c_id, BH)
        tsl = slice(tq * TLOC, (tq + 1) * TLOC)
        bsl = slice(hb * BLOC, (hb + 1) * BLOC)
        fblk = feat[tsl, bsl, :]  # [1024, 128, 128] (t, b, f) fp32
        featT = np.empty((FEAT, TLOC * BLOC), np.float32)
        for j in range(0, TLOC, BLT):
            featT[:, j * BLOC:(j + BLT) * BLOC] = (
                fblk[j:j + BLT].reshape(BLT * BLOC, FEAT).T)
        featT += beta[:, None]
        featT = featT.astype(ml_dtypes.float8_e4m3)
        rt = np.ascontiguousarray(r2[tsl, bsl].T).astype(np.float16)
        in_maps.append({
            "feat": featT, "rt": rt,
            "w1": w1f8, "b2col": b2col,
        })
    return in_maps, npos


def kernel(r, feat, W1, b1, W2, b2, _run_kwargs=None, _return_results=False):
    from concourse.bass_utils import run_bass_kernel_spmd

    r = np.asarray(r, dtype=np.float32)
    feat = np.asarray(feat, dtype=np.float32)

    in_maps, npos = _host_prep(r, feat, W1, b1, W2, b2)
    nc = _get_program(npos)

    kw = _run_kwargs or {}
    res = run_bass_kernel_spmd(nc, in_maps, core_ids=list(range(NCORES)), **kw)

    # host stitch: y = z + P*carry per slab, carry chain across t-quarters
    y = np.empty((T, B), dtype=np.float32)
    for hb in range(BH):
        bsl = slice(hb * BLOC, (hb + 1) * BLOC)
        carry = r[0, bsl, 0].astype(np.float32)
        for tq in range(TQ):
            c_id = tq * BH + hb
            zc = res.results[c_id]["z"].astype(np.float32).T  # [TLOC, BLOC]
            pc = res.results[c_id]["p"].astype(np.float32).T
            y_slab = zc + pc * carry[None, :]
            carry = y_slab[-1]
            y[tq * TLOC:(tq + 1) * TLOC, bsl] = y_slab
    out = y[:, :, None]
    if _return_results:
        return out, res
    return out


# revision 13
# speedup vs baseline: 1.2426x; 1.0094x over previous
"""EMA head kernel for Trainium2 (Bass/Tile), 8 NeuronCores.

Problem: alpha = clip(sigmoid(MLP(feat)), 0.01, 0.99) per (t, b);
         y[0] = r[0]; y[t] = (1-alpha[t])*y[t-1] + alpha[t]*r[t].

Sharding v13: 2 batch-halves x 4 time-quarters.  Each core handles
128 b-lanes (full partition dim) x 1024 t.  Per-core affine-scan
pieces z/P; host stitches slabs with y = z + P*carry.

Math: W2, b1 and the layer-1 bias are all folded away on the host:
  w1f[:, h'] = S * |w2[h']| * W1[:, h']   (S = 16, h' sign-sorted)
  feat' = feat + beta  where  w1f.T beta = S*|w2|*b1  (least-norm)
so that  sum_h w2*relu(x+b1) = (sum_pos relu(y) - sum_neg relu(y))/S
with y = feat' @ w1f.  The per-block epilogue is then one ACT Relu
(PSUM fp32 -> SBUF fp16) and two DVE reduces (pos/neg column groups
into apreP/apreN); a per-segment subtract + sigmoid(bias=b2,
scale=1/S) recovers alpha.

feat is fp8(e4m3), host-pretransposed to [f, t*128b]; everything
vector-side is fp16 (scan state is fp32 internally).  feat streams on
the sync queue; z/p go out per segment, issues deferred on the sync
queue so they never block feat chunk issues.
"""

import numpy as np

T, B, FEAT, HID = 4096, 256, 128, 16
NCORES = 8
TQ, BH = 4, 2        # time-quarters x batch-halves
TLOC = T // TQ       # 1024 t per core
BLOC = B // BH       # 128 b per core
S = 16.0             # w1 fold scale
CH_T = 128           # t-steps per feat chunk (2 MB fp8)
NCHUNK = TLOC // CH_T  # 8
TBLK = 64            # t-steps per PSUM block (2 banks)
SEG = 256            # t-steps per scan/tail segment

_CACHE = {}


def _build_program(npos):
    import concourse.bacc as bacc
    import concourse.tile as tile
    from concourse import mybir

    fp32 = mybir.dt.float32
    fp16 = mybir.dt.float16
    fp8 = mybir.dt.float8e4
    AF = mybir.ActivationFunctionType
    OP = mybir.AluOpType

    nc = bacc.Bacc("TRN2", target_bir_lowering=False, debug=False,
                   num_devices=NCORES)

    feat_d = nc.dram_tensor("feat", [FEAT, TLOC * BLOC], fp8,
                            kind="ExternalInput")
    rt_d = nc.dram_tensor("rt", [BLOC, TLOC], fp16, kind="ExternalInput")
    w1_d = nc.dram_tensor("w1", [FEAT, HID], fp8, kind="ExternalInput")
    b2col_d = nc.dram_tensor("b2col", [BLOC, 1], fp32, kind="ExternalInput")
    z_d = nc.dram_tensor("z", [BLOC, TLOC], fp16, kind="ExternalOutput")
    p_d = nc.dram_tensor("p", [BLOC, TLOC], fp16, kind="ExternalOutput")

    with tile.TileContext(nc) as tc:
        with (
            tc.tile_pool(name="singles", bufs=1) as singles,
            tc.tile_pool(name="featin", bufs=5) as featin,
            tc.tile_pool(name="hps", bufs=3, space="PSUM") as hps,
            tc.tile_pool(name="hwork", bufs=3) as hwork,
        ):
            # first feat chunk DMA before anything else (shortest lead-in);
            # tapered sub-splits so the first 64-t block is ready earliest
            ft0 = featin.tile([128, CH_T * BLOC], fp8, tag="ft")
            for lo, hi in ((0, 16), (16, 32), (32, 64), (64, 128)):
                nc.sync.dma_start(ft0[:, lo * BLOC:hi * BLOC],
                                  feat_d[:, lo * BLOC:hi * BLOC])

            # constants / small inputs on the scalar queue
            w1_sb = singles.tile([128, HID], fp8)
            nc.scalar.dma_start(w1_sb, w1_d[:, :])
            b2col = singles.tile([128, 1], fp32)
            nc.scalar.dma_start(b2col, b2col_d[:, :])
            rt_sb = singles.tile([128, TLOC], fp16)
            nc.scalar.dma_start(rt_sb, rt_d[:, :])
            ones_sb = singles.tile([128, TLOC], fp16)
            nc.vector.memset(ones_sb, 1.0)

            apreP = singles.tile([128, TLOC], fp16, name="apreP")
            apreN = singles.tile([128, TLOC], fp16, name="apreN")
            if npos == HID:
                nc.vector.memset(apreN, 0.0)
            if npos == 0:
                nc.vector.memset(apreP, 0.0)
            alpha = singles.tile([128, TLOC], fp16, name="alpha")
            A_sb = singles.tile([128, TLOC], fp16, name="A")
            Bv = singles.tile([128, TLOC], fp16, name="Bv")
            z_sb = singles.tile([128, TLOC], fp16, name="z")
            p_sb = singles.tile([128, TLOC], fp16, name="p")

            zp_out = []  # deferred z/p DMAs (sync queue, after ft issues)

            def tail_segment(lo, hi):
                sl = slice(lo, hi)
                nc.vector.tensor_sub(apreP[:, sl], apreP[:, sl],
                                     apreN[:, sl])
                nc.scalar.activation(alpha[:, sl], apreP[:, sl], AF.Sigmoid,
                                     bias=b2col, scale=1.0 / S)
                nc.vector.tensor_scalar(alpha[:, sl], alpha[:, sl],
                                        0.01, 0.99, op0=OP.max, op1=OP.min)
                nc.gpsimd.tensor_scalar(A_sb[:, sl], alpha[:, sl],
                                        -1.0, 1.0, op0=OP.mult, op1=OP.add)
                nc.gpsimd.tensor_mul(Bv[:, sl], alpha[:, sl], rt_sb[:, sl])
                nc.vector.tensor_tensor_scan(
                    z_sb[:, sl], A_sb[:, sl], Bv[:, sl],
                    0.0 if lo == 0 else z_sb[:, lo - 1:lo],
                    op0=OP.mult, op1=OP.add)
                nc.vector.tensor_tensor_scan(
                    p_sb[:, sl], A_sb[:, sl], ones_sb[:, sl],
                    1.0 if lo == 0 else p_sb[:, lo - 1:lo],
                    op0=OP.mult, op1=OP.mult)
                zp_out.append(sl)

            # 128-t chunks, tapering to 64 t at the end so less epilogue
            # work remains once the feat stream finishes
            chunk_t = [CH_T] * 7 + [TBLK] * 2
            offs = np.cumsum([0] + chunk_t).tolist()
            for k, (t_lo, ct) in enumerate(zip(offs[:-1], chunk_t)):
                if k == 0:
                    ft = ft0
                else:
                    ft = featin.tile([128, CH_T * BLOC], fp8, tag="ft")
                    nsub = 2 if ct == CH_T else 1
                    sub = ct * BLOC // nsub
                    for q in range(nsub):
                        nc.sync.dma_start(
                            ft[:, q * sub:(q + 1) * sub],
                            feat_d[:, t_lo * BLOC + q * sub:
                                   t_lo * BLOC + (q + 1) * sub])
                for blk in range(ct // TBLK):
                    hbank = hps.tile([128, TBLK, HID], fp32, name="hbank")
                    for tt in range(TBLK):
                        col = (blk * TBLK + tt) * BLOC
                        nc.tensor.matmul(
                            hbank[:, tt, :], ft[:, col:col + BLOC], w1_sb,
                            start=True, stop=True, skip_group_check=True)
                    # ACT applies relu while converting PSUM fp32 -> fp16;
                    # DVE then sums the pos / neg column groups.
                    hw = hwork.tile([128, TBLK, HID], fp16, tag="hw")
                    nc.scalar.activation(hw, hbank, AF.Relu)
                    t0 = t_lo + blk * TBLK
                    with nc.allow_low_precision(
                            "fp16 apre validated vs numpy, 16-elem sums"):
                        if npos > 0:
                            nc.vector.tensor_reduce(
                                apreP[:, t0:t0 + TBLK], hw[:, :, :npos],
                                axis=mybir.AxisListType.X, op=OP.add)
                        if npos < HID:
                            nc.vector.tensor_reduce(
                                apreN[:, t0:t0 + TBLK], hw[:, :, npos:],
                                axis=mybir.AxisListType.X, op=OP.add)
                # coarse 256-t tails early (less DVE overhead), fine
                # 128/64-t tails at the end (short post-DMA drain)
                tails = {2: (0, 256), 4: (256, 512), 6: (512, 768),
                         7: (768, 896), 8: (896, 960)}
                if k in tails:
                    tail_segment(*tails[k])
            # deferred z/p output DMAs: on the sync queue, after every feat
            # chunk issue so they can never stall the feat stream
            for sl in zp_out:
                nc.sync.dma_start(z_d[:, sl], z_sb[:, sl])
                nc.sync.dma_start(p_d[:, sl], p_sb[:, sl])
            tail_segment(offs[-2], offs[-1])
            sl = zp_out[-1]
            nc.sync.dma_start(z_d[:, sl], z_sb[:, sl])
            nc.sync.dma_start(p_d[:, sl], p_sb[:, sl])

    nc.finalize()
    return nc


def _get_program(npos):
    key = ("nc", npos)
    if key not in _CACHE:
        _CACHE[key] = _build_program(npos)
    return _CACHE[key]


def _host_prep(r, feat, W1, b1, W2, b2):
    import ml_dtypes
    W1 = np.asarray(W1, dtype=np.float32)
    b1 = np.asarray(b1, dtype=np.float32).reshape(HID)
    W2 = np.asarray(W2, dtype=np.float32).reshape(HID)
    b2 = float(np.asarray(b2, dtype=np.float32).reshape(1)[0])

    perm = np.argsort(W2 < 0, kind="stable")
    w2s, b1s = W2[perm], b1[perm]
    npos = int((w2s >= 0).sum())

    w1f8 = (S * np.abs(w2s)[None, :] * W1[:, perm]).astype(
        ml_dtypes.float8_e4m3)
    w1fq = w1f8.astype(np.float64)  # dequantized, for the bias solve
    d = (S * np.abs(w2s) * b1s).astype(np.float64)
    beta = np.linalg.lstsq(w1fq.T, d, rcond=None)[0].astype(np.float32)

    b2col = np.full((BLOC, 1), b2, dtype=np.float32)

    r2 = r[:, :, 0]
    in_maps = []
    BLT = 32  # t rows per transpose block (1 MB window)
    for c_id in range(NCORES):
        tq, hb = divmod(c_id, BH)
        tsl = slice(tq * TLOC, (tq + 1) * TLOC)
        bsl = slice(hb * BLOC, (hb + 1) * BLOC)
        fblk = feat[tsl, bsl, :]  # [1024, 128, 128] (t, b, f) fp32
        featT = np.empty((FEAT, TLOC * BLOC), np.float32)
        for j in range(0, TLOC, BLT):
            featT[:, j * BLOC:(j + BLT) * BLOC] = (
                fblk[j:j + BLT].reshape(BLT * BLOC, FEAT).T)
        featT += beta[:, None]
        featT = featT.astype(ml_dtypes.float8_e4m3)
        rt = np.ascontiguousarray(r2[tsl, bsl].T).astype(np.float16)
        in_maps.append({
            "feat": featT, "rt": rt,
            "w1": w1f8, "b2col": b2col,
        })
    return in_maps, npos


def kernel(r, feat, W1, b1, W2, b2, _run_kwargs=None, _return_results=False):
    from concourse.bass_utils import run_bass_kernel_spmd

    r = np.asarray(r, dtype=np.float32)
    feat = np.asarray(feat, dtype=np.float32)

    in_maps, npos = _host_prep(r, feat, W1, b1, W2, b2)
    nc = _get_program(npos)

    kw = _run_kwargs or {}
    res = run_bass_kernel_spmd(nc, in_maps, core_ids=list(range(NCORES)), **kw)

    # host stitch: y = z + P*carry per slab, carry chain across t-quarters
    y = np.empty((T, B), dtype=np.float32)
    for hb in range(BH):
        bsl = slice(hb * BLOC, (hb + 1) * BLOC)
        carry = r[0, bsl, 0].astype(np.float32)
        for tq in range(TQ):
            c_id = tq * BH + hb
            zc = res.results[c_id]["z"].astype(np.float32).T  # [TLOC, BLOC]
            pc = res.results[c_id]["p"].astype(np.float32).T
            y_slab = zc + pc * carry[None, :]
            carry = y_slab[-1]
            y[tq * TLOC:(tq + 1) * TLOC, bsl] = y_slab
    out = y[:, :, None]
    if _return_results:
        return out, res
    return out
